# revision 1
# baseline (speedup 1.0000x reference)
"""CrossLayerTranscoder TRN2 kernel: 8-core data-parallel over the batch dim.

B=4096, L=12, A=768, F=4096. Each NeuronCore gets a 512-row batch shard;
parameters are replicated. Per core:

  encoder  enc[l] = x[l] @ (W_enc[l]/std[l]) + bias[l]   (fp16 MMs, fp32 PSUM)
  jumprelu act    = enc * (enc > theta)                  (one DVE op per tile)
  decoder  logits[i] = sum_{j<=i} act[j] @ W_dec[tri(i)+j]

All matmuls keep the contraction dim on SBUF partitions:
  encoder: lhsT = W_enc[a,f] tile, rhs = x^T[a,b] -> enc^T[f,b]
  decoder: lhsT = W_dec[f,a] tile, rhs = act^T[f,b] -> logits^T[a,b]
so theta/bias/std are per-partition scalars. The input standardization is
folded into the encoder weights on the host (W/std and a bias row); outputs
are produced transposed ([f,b] / [a,b]) and transposed back on the host.
act for layers j < C_CACHE stays resident in SBUF; later layers round-trip
through an internal DRAM buffer.
"""

import os

import numpy as np

B, L, A, F = 4096, 12, 768, 4096
NC = 8
BC = B // NC
AT, FT = A // 128, F // 128
N_TRI = L * (L + 1) // 2
C_CACHE = 2
FGRP = 8

_compiled = None


def _build():
    from contextlib import ExitStack

    import concourse.bacc as bacc
    import concourse.mybir as mybir
    import concourse.tile as tile

    f16 = mybir.dt.float16
    f32 = mybir.dt.float32

    nc = bacc.Bacc("TRN2", target_bir_lowering=False, debug=False)

    x_t = nc.dram_tensor("x_t", [L, A, BC], f16, kind="ExternalInput")
    w_enc = nc.dram_tensor("w_enc", [L, A, F], f16, kind="ExternalInput")
    w_dec = nc.dram_tensor("w_dec", [N_TRI, F, A], f16, kind="ExternalInput")
    theta_r = nc.dram_tensor("theta_r", [128, FT * L], f32, kind="ExternalInput")
    bias_r = nc.dram_tensor("bias_r", [128, FT * L], f32, kind="ExternalInput")
    enc_out = nc.dram_tensor("enc_out", [L, FT, 128, BC], f32, kind="ExternalOutput")
    logits_out = nc.dram_tensor("logits_out", [L, AT, 128, BC], f32,
                                kind="ExternalOutput")

    with tile.TileContext(nc) as tc, ExitStack() as ctx:
        const = ctx.enter_context(tc.tile_pool(name="const", bufs=1))
        xpool = ctx.enter_context(tc.tile_pool(name="xpool", bufs=2 * AT))
        wencp = ctx.enter_context(tc.tile_pool(name="wencp", bufs=2 * AT))
        wdecp = ctx.enter_context(tc.tile_pool(name="wdecp", bufs=4))
        actin = ctx.enter_context(tc.tile_pool(name="actin", bufs=4))
        acache = ctx.enter_context(tc.tile_pool(name="acache", bufs=1))
        encsb = ctx.enter_context(tc.tile_pool(name="encsb", bufs=4))
        actsb = ctx.enter_context(tc.tile_pool(name="actsb", bufs=4))
        loutp = ctx.enter_context(tc.tile_pool(name="loutp", bufs=AT))
        dramp = ctx.enter_context(tc.tile_pool(name="dramp", bufs=1, space="DRAM"))
        encps = ctx.enter_context(tc.tile_pool(name="encps", bufs=2, space="PSUM"))
        decps = ctx.enter_context(tc.tile_pool(name="decps", bufs=1, space="PSUM"))

        th_sb = const.tile([128, FT * L], f32)
        bi_sb = const.tile([128, FT * L], f32)
        nc.sync.dma_start(th_sb[:], theta_r[:])
        nc.sync.dma_start(bi_sb[:], bias_r[:])

        act_dram = dramp.tile([L, FT, 128, BC], f16)
        cache_tiles = [
            acache.tile([128, FT, BC], f16, name=f"acache{j}", tag=f"acache{j}")
            for j in range(C_CACHE)
        ]

        Ident = mybir.ActivationFunctionType.Identity
        is_gt, mult = mybir.AluOpType.is_gt, mybir.AluOpType.mult

        # ---------------- encoder ----------------
        for l in range(L):
            xs = []
            for a_t in range(AT):
                xt = xpool.tile([128, BC], f16, name=f"x_{l}_{a_t}", tag="x")
                nc.sync.dma_start(xt[:], x_t[l, a_t * 128:(a_t + 1) * 128, :])
                xs.append(xt)
            for g in range(FT // FGRP):
                ws = []
                for a_t in range(AT):
                    wt = wencp.tile([128, FGRP * 128], f16,
                                    name=f"we_{l}_{g}_{a_t}", tag="wenc")
                    nc.sync.dma_start(
                        wt[:], w_enc[l, a_t * 128:(a_t + 1) * 128,
                                     g * FGRP * 128:(g + 1) * FGRP * 128])
                    ws.append(wt)
                for fi in range(FGRP):
                    f_t = g * FGRP + fi
                    ps = encps.tile([128, BC], f32, name=f"eps_{l}_{f_t}", tag="eps")
                    for a_t in range(AT):
                        nc.tensor.matmul(ps[:], ws[a_t][:, fi * 128:(fi + 1) * 128],
                                         xs[a_t][:], start=(a_t == 0),
                                         stop=(a_t == AT - 1))
                    idx = f_t * L + l
                    enc_t = encsb.tile([128, BC], f32, name=f"enc_{l}_{f_t}", tag="enc")
                    nc.scalar.activation(enc_t[:], ps[:], Ident,
                                         bias=bi_sb[:, idx:idx + 1], scale=1.0)
                    nc.sync.dma_start(enc_out[l, f_t], enc_t[:])
                    if l < C_CACHE:
                        act_t = cache_tiles[l][:, f_t, :]
                    else:
                        act_t = actsb.tile([128, BC], f16,
                                           name=f"act_{l}_{f_t}", tag="act")
                    nc.vector.scalar_tensor_tensor(
                        act_t, enc_t[:], th_sb[:, idx:idx + 1], enc_t[:], is_gt, mult)
                    if l >= C_CACHE:
                        nc.sync.dma_start(act_dram[l, f_t], act_t)

        # ---------------- decoder ----------------
        for i in range(L):
            base = i * (i + 1) // 2
            pss = [decps.tile([128, BC], f32, name=f"dps_{i}_{a}", tag=f"dps{a}")
                   for a in range(AT)]
            for j in range(i + 1):
                for f_t in range(FT):
                    wd = wdecp.tile([128, A], f16, name=f"wd_{i}_{j}_{f_t}", tag="wd")
                    nc.sync.dma_start(wd[:],
                                      w_dec[base + j, f_t * 128:(f_t + 1) * 128, :])
                    if j < C_CACHE:
                        rhs = cache_tiles[j][:, f_t, :]
                    else:
                        ai = actin.tile([128, BC], f16,
                                        name=f"ai_{i}_{j}_{f_t}", tag="ai")
                        nc.sync.dma_start(ai[:], act_dram[j, f_t])
                        rhs = ai
                    first = (j == 0 and f_t == 0)
                    last = (j == i and f_t == FT - 1)
                    for a_t in range(AT):
                        nc.tensor.matmul(pss[a_t][:],
                                         wd[:, a_t * 128:(a_t + 1) * 128],
                                         rhs, start=first, stop=last)
            for a_t in range(AT):
                lo = loutp.tile([128, BC], f32, name=f"lo_{i}_{a_t}", tag="lo")
                nc.any.tensor_copy(out=lo[:], in_=pss[a_t][:])
                nc.sync.dma_start(logits_out[i, a_t], lo[:])

    nc.compile()
    return nc


def kernel(x, in_mean, in_std, W_enc, theta, W_dec_packed):
    global _compiled
    from concourse.bass_utils import run_bass_kernel_spmd

    # ---- host prep: fold standardization into the encoder weights ----
    w_enc16 = (W_enc / in_std[:, :, None]).astype(np.float16)          # [L, A, F]
    bias = -np.einsum("la,laf->lf", in_mean.astype(np.float32),
                      w_enc16.astype(np.float32), optimize=True)       # [L, F]
    w_dec16 = W_dec_packed.astype(np.float16)                          # [n_tri, F, A]
    theta_r = np.ascontiguousarray(
        theta.T.reshape(FT, 128, L).transpose(1, 0, 2)).reshape(128, FT * L)
    bias_r = np.ascontiguousarray(
        bias.T.reshape(FT, 128, L).transpose(1, 0, 2)).reshape(128, FT * L)
    theta_r = theta_r.astype(np.float32)
    bias_r = bias_r.astype(np.float32)

    if _compiled is None:
        _compiled = _build()
    nc = _compiled

    in_maps = []
    for c in range(NC):
        x_c = np.ascontiguousarray(
            x[c * BC:(c + 1) * BC].transpose(1, 2, 0)).astype(np.float16)
        in_maps.append({"x_t": x_c, "w_enc": w_enc16, "w_dec": w_dec16,
                        "theta_r": theta_r, "bias_r": bias_r})

    trace = bool(int(os.environ.get("CLT_TRACE", "0")))
    res = run_bass_kernel_spmd(nc, in_maps, list(range(NC)), trace=trace)
    if trace and res.exec_time_ns is not None:
        print(f"HW exec time: {res.exec_time_ns} ns")
        kernel.last_exec_time_ns = res.exec_time_ns
        kernel.last_trace = res.instructions_and_trace

    logits = np.empty((B, L, A), np.float32)
    enc = np.empty((B, L, F), np.float32)
    for c in range(NC):
        r = res.results[c]
        enc[c * BC:(c + 1) * BC] = (
            r["enc_out"].reshape(L, F, BC).transpose(2, 0, 1))
        logits[c * BC:(c + 1) * BC] = (
            r["logits_out"].reshape(L, A, BC).transpose(2, 0, 1))
    return logits, enc


# revision 7
# speedup vs baseline: 1.2124x; 1.2124x over previous
"""CrossLayerTranscoder TRN2 kernel: 8-core data-parallel over the batch dim.

B=4096, L=12, A=768, F=4096. Each NeuronCore gets a 512-row batch shard;
parameters are replicated. Per core:

  encoder  enc[l] = x[l] @ (W_enc[l]/std[l]) + bias[l]   (fp16 MMs, fp32 PSUM)
  jumprelu act    = enc * (enc > theta)                  (one DVE op per tile)
  decoder  logits[i] = sum_{j<=i} act[j] @ W_dec[tri(i)+j]

All matmuls keep the contraction dim on SBUF partitions:
  encoder: lhsT = W_enc[a,f] tile, rhs = x^T[a,b] -> enc^T[f,b]
  decoder: lhsT = W_dec[f,a] tile, rhs = act^T[f,b] -> logits^T[a,b]
so theta/bias/std are per-partition scalars. The input standardization is
folded into the encoder weights on the host (W/std and a bias row); outputs
are produced transposed ([f,b] / [a,b]) and transposed back on the host.
act for layers j < C_CACHE stays resident in SBUF; later layers round-trip
through an internal DRAM buffer.
"""

import os

import numpy as np

B, L, A, F = 4096, 12, 768, 4096
NC = 8
BC = B // NC
AT, FT = A // 128, F // 128
N_TRI = L * (L + 1) // 2
C_CACHE = 2
FGRP = 8
DQ = 4  # f-tiles packed per DRAM row for decoder weight / act streams
FG = FT // DQ

_compiled = None


def _build():
    from contextlib import ExitStack

    import concourse.bacc as bacc
    import concourse.mybir as mybir
    import concourse.tile as tile

    f16 = mybir.dt.float16
    f32 = mybir.dt.float32

    nc = bacc.Bacc("TRN2", target_bir_lowering=False, debug=False)

    x_t = nc.dram_tensor("x_t", [L, A, BC], f16, kind="ExternalInput")
    w_enc = nc.dram_tensor("w_enc", [L, A, F], f16, kind="ExternalInput")
    # [t, fg, p, q, a]: f-tile index = fg*DQ + q, partition p within tile.
    # 4 f-tiles share one 6KB DRAM row -> large DMA bursts.
    w_dec = nc.dram_tensor("w_dec", [N_TRI, FG, 128, DQ * A], f16,
                           kind="ExternalInput")
    theta_r = nc.dram_tensor("theta_r", [128, FT * L], f32, kind="ExternalInput")
    bias_r = nc.dram_tensor("bias_r", [128, FT * L], f32, kind="ExternalInput")
    enc_out = nc.dram_tensor("enc_out", [L, FT, 128, BC], f32, kind="ExternalOutput")
    logits_out = nc.dram_tensor("logits_out", [L, AT, 128, BC], f32,
                                kind="ExternalOutput")

    with tile.TileContext(nc) as tc, ExitStack() as ctx:
        const = ctx.enter_context(tc.tile_pool(name="const", bufs=1))
        xpool = ctx.enter_context(tc.tile_pool(name="xpool", bufs=2 * AT))
        wencp = ctx.enter_context(tc.tile_pool(name="wencp", bufs=2 * AT))
        wdecp = ctx.enter_context(tc.tile_pool(name="wdecp", bufs=4))
        actin = ctx.enter_context(tc.tile_pool(name="actin", bufs=4))
        acache = ctx.enter_context(tc.tile_pool(name="acache", bufs=1))
        encsb = ctx.enter_context(tc.tile_pool(name="encsb", bufs=4))
        actsb = ctx.enter_context(tc.tile_pool(name="actsb", bufs=4))
        loutp = ctx.enter_context(tc.tile_pool(name="loutp", bufs=AT))
        dramp = ctx.enter_context(tc.tile_pool(name="dramp", bufs=1, space="DRAM"))
        encps = ctx.enter_context(tc.tile_pool(name="encps", bufs=2, space="PSUM"))
        decps = ctx.enter_context(tc.tile_pool(name="decps", bufs=1, space="PSUM"))

        th_sb = const.tile([128, FT * L], f32)
        bi_sb = const.tile([128, FT * L], f32)
        nc.sync.dma_start(th_sb[:], theta_r[:])
        nc.sync.dma_start(bi_sb[:], bias_r[:])

        act_dram = dramp.tile([L, FG, 128, DQ, BC], f16)
        cache_tiles = [
            acache.tile([128, FT, BC], f16, name=f"acache{j}", tag=f"acache{j}")
            for j in range(C_CACHE)
        ]

        Ident = mybir.ActivationFunctionType.Identity
        is_gt, mult = mybir.AluOpType.is_gt, mybir.AluOpType.mult

        # ---------------- encoder ----------------
        for l in range(L):
            xs = []
            for a_t in range(AT):
                xt = xpool.tile([128, BC], f16, name=f"x_{l}_{a_t}", tag="x")
                nc.sync.dma_start(xt[:], x_t[l, a_t * 128:(a_t + 1) * 128, :])
                xs.append(xt)
            for g in range(FT // FGRP):
                ws = []
                for a_t in range(AT):
                    wt = wencp.tile([128, FGRP * 128], f16,
                                    name=f"we_{l}_{g}_{a_t}", tag="wenc")
                    nc.sync.dma_start(
                        wt[:], w_enc[l, a_t * 128:(a_t + 1) * 128,
                                     g * FGRP * 128:(g + 1) * FGRP * 128])
                    ws.append(wt)
                stage = None
                for fi in range(FGRP):
                    f_t = g * FGRP + fi
                    fg, q = f_t // DQ, f_t % DQ
                    ps = encps.tile([128, BC], f32, name=f"eps_{l}_{f_t}", tag="eps")
                    for a_t in range(AT):
                        nc.tensor.matmul(ps[:], ws[a_t][:, fi * 128:(fi + 1) * 128],
                                         xs[a_t][:], start=(a_t == 0),
                                         stop=(a_t == AT - 1))
                    idx = f_t * L + l
                    enc_t = encsb.tile([128, BC], f32, name=f"enc_{l}_{f_t}", tag="enc")
                    nc.scalar.activation(enc_t[:], ps[:], Ident,
                                         bias=bi_sb[:, idx:idx + 1], scale=1.0)
                    nc.sync.dma_start(enc_out[l, f_t], enc_t[:])
                    if l < C_CACHE:
                        act_t = cache_tiles[l][:, f_t, :]
                    else:
                        if q == 0:
                            stage = actsb.tile([128, DQ, BC], f16,
                                               name=f"act_{l}_{fg}", tag="act")
                        act_t = stage[:, q, :]
                    nc.vector.scalar_tensor_tensor(
                        act_t, enc_t[:], th_sb[:, idx:idx + 1], enc_t[:], is_gt, mult)
                    if l >= C_CACHE and q == DQ - 1:
                        nc.sync.dma_start(act_dram[l, fg], stage[:])

        # ---------------- decoder ----------------
        for i in range(L):
            base = i * (i + 1) // 2
            pss = [decps.tile([128, BC], f32, name=f"dps_{i}_{a}", tag=f"dps{a}")
                   for a in range(AT)]
            for j in range(i + 1):
                for fg in range(FG):
                    wd = wdecp.tile([128, DQ * A], f16,
                                    name=f"wd_{i}_{j}_{fg}", tag="wd")
                    nc.sync.dma_start(wd[:], w_dec[base + j, fg])
                    ai = None
                    if j >= C_CACHE:
                        ai = actin.tile([128, DQ, BC], f16,
                                        name=f"ai_{i}_{j}_{fg}", tag="ai")
                        nc.sync.dma_start(ai[:], act_dram[j, fg])
                    for q in range(DQ):
                        f_t = fg * DQ + q
                        if j < C_CACHE:
                            rhs = cache_tiles[j][:, f_t, :]
                        else:
                            rhs = ai[:, q, :]
                        first = (j == 0 and f_t == 0)
                        last = (j == i and f_t == FT - 1)
                        for a_t in range(AT):
                            nc.tensor.matmul(
                                pss[a_t][:],
                                wd[:, q * A + a_t * 128:q * A + (a_t + 1) * 128],
                                rhs, start=first, stop=last)
            for a_t in range(AT):
                lo = loutp.tile([128, BC], f32, name=f"lo_{i}_{a_t}", tag="lo")
                nc.any.tensor_copy(out=lo[:], in_=pss[a_t][:])
                nc.sync.dma_start(logits_out[i, a_t], lo[:])

    nc.compile()
    return nc


def kernel(x, in_mean, in_std, W_enc, theta, W_dec_packed):
    global _compiled
    from concourse.bass_utils import run_bass_kernel_spmd

    # ---- host prep: fold standardization into the encoder weights ----
    w_enc16 = (W_enc / in_std[:, :, None]).astype(np.float16)          # [L, A, F]
    bias = -np.einsum("la,laf->lf", in_mean.astype(np.float32),
                      w_enc16.astype(np.float32), optimize=True)       # [L, F]
    # pack 4 f-tiles per DRAM row: [t, fg, p, q*A] with f = (fg*DQ+q)*128 + p
    w_dec16 = np.ascontiguousarray(
        W_dec_packed.astype(np.float16)
        .reshape(N_TRI, FG, DQ, 128, A)
        .transpose(0, 1, 3, 2, 4)).reshape(N_TRI, FG, 128, DQ * A)
    theta_r = np.ascontiguousarray(
        theta.T.reshape(FT, 128, L).transpose(1, 0, 2)).reshape(128, FT * L)
    bias_r = np.ascontiguousarray(
        bias.T.reshape(FT, 128, L).transpose(1, 0, 2)).reshape(128, FT * L)
    theta_r = theta_r.astype(np.float32)
    bias_r = bias_r.astype(np.float32)

    if _compiled is None:
        _compiled = _build()
    nc = _compiled

    in_maps = []
    for c in range(NC):
        x_c = np.ascontiguousarray(
            x[c * BC:(c + 1) * BC].transpose(1, 2, 0)).astype(np.float16)
        in_maps.append({"x_t": x_c, "w_enc": w_enc16, "w_dec": w_dec16,
                        "theta_r": theta_r, "bias_r": bias_r})

    trace = bool(int(os.environ.get("CLT_TRACE", "0")))
    res = run_bass_kernel_spmd(nc, in_maps, list(range(NC)), trace=trace)
    if trace and res.exec_time_ns is not None:
        print(f"HW exec time: {res.exec_time_ns} ns")
        kernel.last_exec_time_ns = res.exec_time_ns
        kernel.last_trace = res.instructions_and_trace

    logits = np.empty((B, L, A), np.float32)
    enc = np.empty((B, L, F), np.float32)
    for c in range(NC):
        r = res.results[c]
        enc[c * BC:(c + 1) * BC] = (
            r["enc_out"].reshape(L, F, BC).transpose(2, 0, 1))
        logits[c * BC:(c + 1) * BC] = (
            r["logits_out"].reshape(L, A, BC).transpose(2, 0, 1))
    return logits, enc


# revision 10
# speedup vs baseline: 1.2451x; 1.0270x over previous
"""CrossLayerTranscoder TRN2 kernel: 8-core data-parallel over the batch dim.

B=4096, L=12, A=768, F=4096. Each NeuronCore gets a 512-row batch shard;
parameters are replicated. Per core:

  encoder  enc[l] = x[l] @ (W_enc[l]/std[l]) + bias[l]   (fp16 MMs, fp32 PSUM)
  jumprelu act    = enc * (enc > theta)                  (one DVE op per tile)
  decoder  logits[i] = sum_{j<=i} act[j] @ W_dec[tri(i)+j]

All matmuls keep the contraction dim on SBUF partitions:
  encoder: lhsT = W_enc[a,f] tile, rhs = x^T[a,b] -> enc^T[f,b]
  decoder: lhsT = W_dec[f,a] tile, rhs = act^T[f,b] -> logits^T[a,b]
so theta/bias/std are per-partition scalars. The input standardization is
folded into the encoder weights on the host (W/std and a bias row); outputs
are produced transposed ([f,b] / [a,b]) and transposed back on the host.
act for layers j < C_CACHE stays resident in SBUF; later layers round-trip
through an internal DRAM buffer.
"""

import os

import numpy as np

B, L, A, F = 4096, 12, 768, 4096
NC = 8
BC = B // NC
AT, FT = A // 128, F // 128
N_TRI = L * (L + 1) // 2
C_CACHE = 2
FGRP = 8
DQ = 4  # f-tiles packed per DRAM row for decoder weight / act streams
FG = FT // DQ

_compiled = None


def _build():
    from contextlib import ExitStack

    import concourse.bacc as bacc
    import concourse.mybir as mybir
    import concourse.tile as tile

    f16 = mybir.dt.float16
    f32 = mybir.dt.float32

    nc = bacc.Bacc("TRN2", target_bir_lowering=False, debug=False)

    x_t = nc.dram_tensor("x_t", [L, A, BC], f16, kind="ExternalInput")
    w_enc = nc.dram_tensor("w_enc", [L, A, F], f16, kind="ExternalInput")
    # [t, fg, p, q, a]: f-tile index = fg*DQ + q, partition p within tile.
    # 4 f-tiles share one 6KB DRAM row -> large DMA bursts.
    w_dec = nc.dram_tensor("w_dec", [N_TRI, FG, 128, DQ * A], f16,
                           kind="ExternalInput")
    theta_r = nc.dram_tensor("theta_r", [128, FT * L], f32, kind="ExternalInput")
    bias_r = nc.dram_tensor("bias_r", [128, FT * L], f32, kind="ExternalInput")
    enc_out = nc.dram_tensor("enc_out", [L, FT, 128, BC], f32, kind="ExternalOutput")
    logits_out = nc.dram_tensor("logits_out", [L, AT, 128, BC], f32,
                                kind="ExternalOutput")

    with tile.TileContext(nc) as tc, ExitStack() as ctx:
        const = ctx.enter_context(tc.tile_pool(name="const", bufs=1))
        xpool = ctx.enter_context(tc.tile_pool(name="xpool", bufs=2 * AT))
        wencp = ctx.enter_context(tc.tile_pool(name="wencp", bufs=2 * AT))
        wdecp = ctx.enter_context(tc.tile_pool(name="wdecp", bufs=4))
        actin = ctx.enter_context(tc.tile_pool(name="actin", bufs=4))
        acache = ctx.enter_context(tc.tile_pool(name="acache", bufs=1))
        encsb = ctx.enter_context(tc.tile_pool(name="encsb", bufs=4))
        actsb = ctx.enter_context(tc.tile_pool(name="actsb", bufs=4))
        loutp = ctx.enter_context(tc.tile_pool(name="loutp", bufs=AT))
        dramp = ctx.enter_context(tc.tile_pool(name="dramp", bufs=1, space="DRAM"))
        encps = ctx.enter_context(tc.tile_pool(name="encps", bufs=2, space="PSUM"))
        decps = ctx.enter_context(tc.tile_pool(name="decps", bufs=1, space="PSUM"))

        th_sb = const.tile([128, FT * L], f32)
        bi_sb = const.tile([128, FT * L], f32)
        nc.sync.dma_start(th_sb[:], theta_r[:])
        nc.sync.dma_start(bi_sb[:], bias_r[:])

        act_dram = dramp.tile([L, FG, 128, DQ, BC], f16)
        cache_tiles = [
            acache.tile([128, FT, BC], f16, name=f"acache{j}", tag=f"acache{j}")
            for j in range(C_CACHE)
        ]

        Ident = mybir.ActivationFunctionType.Identity
        is_gt, mult = mybir.AluOpType.is_gt, mybir.AluOpType.mult

        # ---------------- per-layer emitters ----------------
        def emit_encoder(l):
            xs = []
            for a_t in range(AT):
                xt = xpool.tile([128, BC], f16, name=f"x_{l}_{a_t}", tag="x")
                nc.sync.dma_start(xt[:], x_t[l, a_t * 128:(a_t + 1) * 128, :])
                xs.append(xt)
            for g in range(FT // FGRP):
                ws = []
                for a_t in range(AT):
                    wt = wencp.tile([128, FGRP * 128], f16,
                                    name=f"we_{l}_{g}_{a_t}", tag="wenc")
                    nc.sync.dma_start(
                        wt[:], w_enc[l, a_t * 128:(a_t + 1) * 128,
                                     g * FGRP * 128:(g + 1) * FGRP * 128])
                    ws.append(wt)
                stage = None
                for fi in range(FGRP):
                    f_t = g * FGRP + fi
                    fg, q = f_t // DQ, f_t % DQ
                    ps = encps.tile([128, BC], f32, name=f"eps_{l}_{f_t}", tag="eps")
                    for a_t in range(AT):
                        nc.tensor.matmul(ps[:], ws[a_t][:, fi * 128:(fi + 1) * 128],
                                         xs[a_t][:], start=(a_t == 0),
                                         stop=(a_t == AT - 1))
                    idx = f_t * L + l
                    enc_t = encsb.tile([128, BC], f32, name=f"enc_{l}_{f_t}", tag="enc")
                    nc.scalar.activation(enc_t[:], ps[:], Ident,
                                         bias=bi_sb[:, idx:idx + 1], scale=1.0)
                    nc.sync.dma_start(enc_out[l, f_t], enc_t[:])
                    if l < C_CACHE:
                        act_t = cache_tiles[l][:, f_t, :]
                    else:
                        if q == 0:
                            stage = actsb.tile([128, DQ, BC], f16,
                                               name=f"act_{l}_{fg}", tag="act")
                        act_t = stage[:, q, :]
                    nc.vector.scalar_tensor_tensor(
                        act_t, enc_t[:], th_sb[:, idx:idx + 1], enc_t[:], is_gt, mult)
                    if l >= C_CACHE and q == DQ - 1:
                        nc.sync.dma_start(act_dram[l, fg], stage[:])

        def emit_decoder(i):
            base = i * (i + 1) // 2
            pss = [decps.tile([128, BC], f32, name=f"dps_{i}_{a}", tag=f"dps{a}")
                   for a in range(AT)]
            for j in range(i + 1):
                for fg in range(FG):
                    wd = wdecp.tile([128, DQ * A], f16,
                                    name=f"wd_{i}_{j}_{fg}", tag="wd")
                    nc.sync.dma_start(wd[:], w_dec[base + j, fg])
                    ai = None
                    if j >= C_CACHE:
                        ai = actin.tile([128, DQ, BC], f16,
                                        name=f"ai_{i}_{j}_{fg}", tag="ai")
                        nc.sync.dma_start(ai[:], act_dram[j, fg])
                    for q in range(DQ):
                        f_t = fg * DQ + q
                        if j < C_CACHE:
                            rhs = cache_tiles[j][:, f_t, :]
                        else:
                            rhs = ai[:, q, :]
                        first = (j == 0 and f_t == 0)
                        last = (j == i and f_t == FT - 1)
                        for a_t in range(AT):
                            nc.tensor.matmul(
                                pss[a_t][:],
                                wd[:, q * A + a_t * 128:q * A + (a_t + 1) * 128],
                                rhs, start=first, stop=last)
            for a_t in range(AT):
                lo = loutp.tile([128, BC], f32, name=f"lo_{i}_{a_t}", tag="lo")
                nc.any.tensor_copy(out=lo[:], in_=pss[a_t][:])
                nc.sync.dma_start(logits_out[i, a_t], lo[:])

        # Interleave E/D emission so decoder matmuls (DMA-light) fill the
        # encoder's DMA-bound stretches in the in-order PE stream:
        # E0, E1, D0, E2, D1, ..., E11, D10, D11.
        for l in range(L):
            emit_encoder(l)
            if l >= 1:
                emit_decoder(l - 1)
        emit_decoder(L - 1)

    nc.compile()
    return nc


def kernel(x, in_mean, in_std, W_enc, theta, W_dec_packed):
    global _compiled
    from concourse.bass_utils import run_bass_kernel_spmd

    # ---- host prep: fold standardization into the encoder weights ----
    w_enc16 = (W_enc / in_std[:, :, None]).astype(np.float16)          # [L, A, F]
    bias = -np.einsum("la,laf->lf", in_mean.astype(np.float32),
                      w_enc16.astype(np.float32), optimize=True)       # [L, F]
    # pack 4 f-tiles per DRAM row: [t, fg, p, q*A] with f = (fg*DQ+q)*128 + p
    w_dec16 = np.ascontiguousarray(
        W_dec_packed.astype(np.float16)
        .reshape(N_TRI, FG, DQ, 128, A)
        .transpose(0, 1, 3, 2, 4)).reshape(N_TRI, FG, 128, DQ * A)
    theta_r = np.ascontiguousarray(
        theta.T.reshape(FT, 128, L).transpose(1, 0, 2)).reshape(128, FT * L)
    bias_r = np.ascontiguousarray(
        bias.T.reshape(FT, 128, L).transpose(1, 0, 2)).reshape(128, FT * L)
    theta_r = theta_r.astype(np.float32)
    bias_r = bias_r.astype(np.float32)

    if _compiled is None:
        _compiled = _build()
    nc = _compiled

    in_maps = []
    for c in range(NC):
        x_c = np.ascontiguousarray(
            x[c * BC:(c + 1) * BC].transpose(1, 2, 0)).astype(np.float16)
        in_maps.append({"x_t": x_c, "w_enc": w_enc16, "w_dec": w_dec16,
                        "theta_r": theta_r, "bias_r": bias_r})

    trace = bool(int(os.environ.get("CLT_TRACE", "0")))
    res = run_bass_kernel_spmd(nc, in_maps, list(range(NC)), trace=trace)
    if trace and res.exec_time_ns is not None:
        print(f"HW exec time: {res.exec_time_ns} ns")
        kernel.last_exec_time_ns = res.exec_time_ns
        kernel.last_trace = res.instructions_and_trace

    logits = np.empty((B, L, A), np.float32)
    enc = np.empty((B, L, F), np.float32)
    for c in range(NC):
        r = res.results[c]
        enc[c * BC:(c + 1) * BC] = (
            r["enc_out"].reshape(L, F, BC).transpose(2, 0, 1))
        logits[c * BC:(c + 1) * BC] = (
            r["logits_out"].reshape(L, A, BC).transpose(2, 0, 1))
    return logits, enc
